# revision 4
# baseline (speedup 1.0000x reference)
"""Trainium2 Bass kernel for a 4-layer LSTM classifier (H=16) over 8 NeuronCores.

Strategy: pure data parallel, batch 256 -> 32/core (sharding_hint). Per core:
  phase 1: input projection pre0 = x @ W_ih_l0a^T streamed from HBM in bf16;
           the host pre-transposes x to [I, (t_hi, b, t_lo)] so the
           contraction dim I lands on SBUF partitions with fully contiguous
           DMA rows. PSUM results are copied to SBUF on the (otherwise idle)
           Pool engine and partition-regrouped via SBUF->SBUF DMA (also on the
           Pool/SWDGE queue) into per-stage pre tiles [16,(type,hi,b,tl)].
           Phase-1 work units (DMA/matmul/copy/regroup) are spread ~2 per
           wavefront step so the in-order PE queue never bursts.
  phase 2: wavefront recurrence over (layer, t): at step s layer l computes
           t = s - l. All 4 layers x 4 gate types are computed as ONE psum
           tile [64,(type,b)] via 1 pre-inject matmul + 4 gate matmuls
           against h_all [65, BW] (input, recurrent and bias terms folded
           into lhsT). ALL gate activations are a single Sigmoid instruction:
           tanh(x) = 2*sigmoid(2x) - 1 is used for the g gate (the 2x is
           folded into the host-side g weights), and h is stored HALVED
           (h/2 = (sigmoid(2c) - .5) * sigmoid(o_hat)) with the compensating
           2x folded into all h-consuming weights. The c update is 3 DVE ops
           (two scalar_tensor_tensor + one tensor_tensor); tanh(c) becomes a
           second Sigmoid (scale=2). h_all is double-buffered so the state
           write has no PE WAR semaphore. Per step per chain: 5 PE matmuls,
           2 ACT sigmoids, 4 DVE ops -- the serial cycle is
           PE -> sigmoid -> DVE*3 -> sigmoid -> DVE.
           The batch is split into 2 phase-offset chains to fill gaps.
  phase 3: FC1(16->16)+ReLU via a select-folded matmul reading h3 rows of
           h_all (doubled host-side), FC2(16->15) with bias folded via a ones
           row, softmax (negated reduce_max as Exp bias, accum_out for the
           sum), DMA out [32, 15] per core; host concatenates to [256, 15].
"""

import sys

if "/opt/trn_rl_repo" not in sys.path:
    sys.path.insert(0, "/opt/trn_rl_repo")

import numpy as np

# ---- problem constants (hardcoded per contract) ----
B, T, I, H, C = 256, 200, 1086, 16, 15
NCORES = 8
BL = B // NCORES          # 32 batch per core
TL = 8                    # t-interleave factor
THI = T // TL             # 25
NCOLS = BL * T            # 6400
STAGE = 512               # phase-1 col stage (= 2 t_hi blocks = 16 t)
NSTAGE = (NCOLS + STAGE - 1) // STAGE  # 13 (last = 256)
KCH = [128] * 8 + [62]    # 1086 contraction chunks
NSTEP = T + 3             # 203 wavefront steps

CFG = dict(
    x_dtype="bfloat16",    # or "float32"
    rec_dtype="bfloat16",  # recurrence state/gate dtype
    nchains=2,             # independent phase-offset recurrence chains
)

_BUILD_CACHE = {}


def _np_dt(name):
    import ml_dtypes
    return np.dtype(ml_dtypes.bfloat16) if name == "bfloat16" else np.dtype(name)


def _gate_rows(w):
    # torch gate row order in 4H matrices: i, f, g, o
    return dict(i=w[0:H], f=w[H:2 * H], g=w[2 * H:3 * H], o=w[3 * H:4 * H])


TYPES = ["i", "f", "o", "g"]  # gate-type order used everywhere on-chip


def build_host_constants(wd, x_dtype, rec_dtype="float32"):
    f32 = np.float32
    # phase-1 W: rows I, cols 64 = (type-major: i0,f0,o0,g0) x16 units
    g0 = _gate_rows(wd["w_ih_l0a"])
    W_proj = np.zeros((I, 64), f32)
    for j, t in enumerate(TYPES):
        W_proj[:, 16 * j:16 * j + 16] = g0[t].T
    W_proj[:, 48:64] *= 2.0            # g gate: sigmoid(2*ghat) trick
    W_proj = W_proj.astype(_np_dt(x_dtype))

    # recurrence weights: per gate type, lhsT [65, 64]
    # h_all rows: h0 0:16, h1 16:32, h2 32:48, h3 48:64, ONE 64
    # cols: unit m = 16*l + u
    hh = [_gate_rows(wd["w_hh_l0a"]), _gate_rows(wd["w_hh_l0b"]),
          _gate_rows(wd["w_hh_l1a"]), _gate_rows(wd["w_hh_l1b"])]
    ih = [None, _gate_rows(wd["w_ih_l0b"]), _gate_rows(wd["w_ih_l1a"]),
          _gate_rows(wd["w_ih_l1b"])]
    bb = [_gate_rows(wd["b_l0a"][:, None]), _gate_rows(wd["b_l0b"][:, None]),
          _gate_rows(wd["b_l1a"][:, None]), _gate_rows(wd["b_l1b"][:, None])]
    lhsT = {}
    for t in TYPES:
        M = np.zeros((65, 64), f32)
        for l in range(4):
            cs = slice(16 * l, 16 * l + 16)
            M[16 * l:16 * l + 16, cs] = hh[l][t].T      # recurrent h_l
            if l >= 1:
                M[16 * (l - 1):16 * l, cs] = ih[l][t].T  # input h_{l-1}
            M[64, cs] = bb[l][t][:, 0]                   # bias
        M[0:64] *= 2.0      # h is stored halved on-chip
        if t == "g":
            M *= 2.0        # sigmoid(2*ghat) trick for tanh
        lhsT[t] = M

    # fc1 folded onto h_all: out1[u,b] = sum_k W1e[k,u] h_all[k,b]
    W1e = np.zeros((65, 16), f32)
    W1e[48:64] = 2.0 * wd["w_fc1"].T    # h3 rows (stored halved)
    W1e[64] = wd["b_fc1"]
    # relu2 tile is [33, BL]: rows 0:16 = relu(fc1), rows 16:32 = zeros,
    # row 32 = ones (32-aligned partition for the memset)
    W2 = np.zeros((33, 15), f32)
    W2[0:16] = wd["w_fc2"].T
    W2[32] = wd["b_fc2"]
    # pre-injection select: maps pre row u -> pg row u (l0 units), zeros rows 16:64
    SEL = np.zeros((16, 64), f32)
    SEL[np.arange(16), np.arange(16)] = 1.0
    SEL = SEL.astype(_np_dt(x_dtype))
    rdt_np = _np_dt(rec_dtype)
    return dict(W_proj=W_proj, lhsT_i=lhsT["i"].astype(rdt_np),
                lhsT_f=lhsT["f"].astype(rdt_np), lhsT_o=lhsT["o"].astype(rdt_np),
                lhsT_g=lhsT["g"].astype(rdt_np), W1e=W1e.astype(rdt_np),
                W2=W2, SEL=SEL)


def build_bass(x_dtype="float32", nchains=2, rec_dtype="float32"):
    from concourse import bacc, mybir

    from concourse.tile import TileContext

    dt = mybir.dt
    xdt = dt.bfloat16 if x_dtype == "bfloat16" else dt.float32
    f32 = dt.float32
    rdt = dt.bfloat16 if rec_dtype == "bfloat16" else dt.float32
    AF = mybir.ActivationFunctionType
    ALU = mybir.AluOpType

    nc = bacc.Bacc("TRN2", target_bir_lowering=False, debug=False,
                   num_devices=NCORES)

    xin = nc.dram_tensor("x", [I, NCOLS], xdt, kind="ExternalInput").ap()
    wproj_d = nc.dram_tensor("wproj", [I, 64], xdt, kind="ExternalInput").ap()
    lhs_d = {t: nc.dram_tensor(f"lhs_{t}", [65, 64], rdt,
                               kind="ExternalInput").ap() for t in TYPES}
    w1_d = nc.dram_tensor("w1", [65, 16], rdt, kind="ExternalInput").ap()
    w2_d = nc.dram_tensor("w2", [33, 15], f32, kind="ExternalInput").ap()
    sel_d = nc.dram_tensor("sel", [16, 64], xdt, kind="ExternalInput").ap()
    out_d = nc.dram_tensor("out", [BL, C], f32, kind="ExternalOutput").ap()

    with TileContext(nc) as tc:
        import contextlib
        with contextlib.ExitStack() as ctx:
            wpool = ctx.enter_context(tc.tile_pool(name="weights", bufs=9))
            xpool = ctx.enter_context(tc.tile_pool(name="xtiles", bufs=18))
            stpool = ctx.enter_context(tc.tile_pool(name="stage", bufs=2))
            prepool = ctx.enter_context(tc.tile_pool(name="pre", bufs=NSTAGE))
            state = ctx.enter_context(tc.tile_pool(name="state", bufs=1))
            work = ctx.enter_context(tc.tile_pool(name="work", bufs=3))
            pg_pool = ctx.enter_context(
                tc.tile_pool(name="pgates", bufs=2, space="PSUM"))
            px_pool = ctx.enter_context(
                tc.tile_pool(name="pproj", bufs=2, space="PSUM"))

            # --- weights ---
            wproj_t = []
            k0 = 0
            for kk in KCH:
                wt = wpool.tile([128, 64], xdt, tag="wproj")
                nc.sync.dma_start(out=wt[0:kk, :], in_=wproj_d[k0:k0 + kk, :])
                wproj_t.append(wt)
                k0 += kk
            lhs = {}
            for t in TYPES:
                lt = wpool.tile([65, 64], rdt, tag=f"lhs_{t}")
                nc.sync.dma_start(out=lt[:], in_=lhs_d[t][:])
                lhs[t] = lt
            w1 = wpool.tile([65, 16], rdt, tag="w1")
            nc.sync.dma_start(out=w1[:], in_=w1_d[:])
            w2 = wpool.tile([33, 15], f32, tag="w2")
            nc.sync.dma_start(out=w2[:], in_=w2_d[:])
            sel = wpool.tile([16, 64], xdt, tag="sel")
            nc.sync.dma_start(out=sel[:], in_=sel_d[:])

            # --- persistent state (one set per chain) ---
            CH = nchains
            BW = BL // CH
            h_bufs, ccs, relu2s = [], [], []
            for c in range(CH):
                hb = []
                for p in range(2):
                    h_all = state.tile([65, BW], rdt, tag=f"h_all{c}_{p}")
                    nc.vector.memset(h_all[:], 0.0)
                    nc.vector.memset(h_all[64:65, :], 1.0)
                    hb.append(h_all)
                h_bufs.append(hb)
                cc = state.tile([64, BW], rdt, tag=f"cc{c}")
                nc.vector.memset(cc[:], 0.0)
                relu2 = state.tile([33, BW], f32, tag=f"relu2{c}")
                nc.vector.memset(relu2[:], 0.0)
                nc.vector.memset(relu2[32:33, :], 1.0)
                ccs.append(cc)
                relu2s.append(relu2)

            # --- phase 1 as a stream of small work units ---
            # pre tile per 512-col stage: [16, type, hi, b, tl]
            pre_tiles = [None] * NSTAGE

            def phase1_units():
                k0s = np.cumsum([0] + KCH)
                xts = [None] * len(KCH)
                for st in range(NSTAGE):
                    c0 = st * STAGE
                    cw = min(STAGE, NCOLS - c0)
                    # x loads for this stage
                    for ki, kk in enumerate(KCH):
                        def load(ki=ki, kk=kk, k0=int(k0s[ki]), c0=c0, cw=cw):
                            xt = xpool.tile([128, STAGE], xdt, tag="xt",
                                            name="xt")
                            nc.sync.dma_start(out=xt[0:kk, 0:cw],
                                              in_=xin[k0:k0 + kk, c0:c0 + cw])
                            xts[ki] = xt
                        yield load
                    px_box = [None]
                    for ki, kk in enumerate(KCH):
                        def mm(ki=ki, kk=kk, cw=cw, px_box=px_box, xts=xts):
                            if ki == 0:
                                px_box[0] = px_pool.tile([64, STAGE], f32,
                                                         tag="px", name="px")
                            nc.tensor.matmul(px_box[0][:, 0:cw],
                                             wproj_t[ki][0:kk, :],
                                             xts[ki][0:kk, 0:cw],
                                             start=(ki == 0),
                                             stop=(ki == len(KCH) - 1))
                        yield mm
                    def stage_copy(st=st, cw=cw, px_box=px_box):
                        stg = stpool.tile([64, STAGE], xdt, tag="stage",
                                          name="stage")
                        nc.gpsimd.tensor_copy(stg[:, 0:cw], px_box[0][:, 0:cw])
                        px_box.append(stg)
                    yield stage_copy
                    nhi = cw // (BL * TL)
                    pt = [None]
                    for j in range(4):
                        def regroup(st=st, j=j, nhi=nhi, px_box=px_box, pt=pt):
                            if j == 0:
                                pt[0] = prepool.tile([16, 4, 2, BL, TL], xdt,
                                                     tag="pre", name="pre")
                                pre_tiles[st] = pt[0]
                            stg = px_box[1]
                            src = stg[16 * j:16 * j + 16, 0:nhi * BL * TL]
                            nc.gpsimd.dma_start(out=pt[0][:, j, 0:nhi, :, :],
                                                in_=src)
                        yield regroup

            # --- recurrence step ---
            def emit_step(s, c):
                cc = ccs[c]
                h_prev = h_bufs[c][(s - 1) % 2]
                h_cur = h_bufs[c][s % 2]
                lmin = max(0, s - (T - 1))
                lmax = min(3, s)
                # write range for state updates; starts must be 32-aligned,
                # so widen r0 down (clobbered rows are only read by inactive
                # layers afterwards -- harmless garbage)
                r0 = (16 * lmin // 32) * 32
                r1 = 16 * (lmax + 1)
                pg = pg_pool.tile([64, 4 * BW], f32, tag=f"pg{c}")
                has_pre = s < T
                if has_pre:
                    st, hi, tl = s // 16, (s // 8) % 2, s % 8
                    pslice = pre_tiles[st][:, :, hi, c * BW:(c + 1) * BW, tl]
                    nc.tensor.matmul(pg[:], sel[:], pslice,
                                     start=True, stop=False,
                                     skip_group_check=True)
                for j, t in enumerate(TYPES):
                    nc.tensor.matmul(pg[:, BW * j:BW * (j + 1)], lhs[t][:],
                                     h_prev[:], start=not has_pre, stop=True,
                                     skip_group_check=True)
                # one sigmoid for all gates: cols (i, f, o, g2)
                sifog = work.tile([64, 4 * BW], rdt, tag=f"sifog{c}")
                nc.scalar.activation(sifog[:], pg[:], AF.Sigmoid)
                # c update: c' = sf*c + 2*(sg - .5)*si
                t1 = work.tile([64, BW], rdt, tag=f"t1{c}")
                nc.vector.scalar_tensor_tensor(
                    t1[:], sifog[:, 3 * BW:4 * BW], 0.5, sifog[:, 0:BW],
                    ALU.subtract, ALU.mult)
                t2 = work.tile([64, BW], rdt, tag=f"t2{c}")
                nc.vector.tensor_tensor(t2[:], sifog[:, BW:2 * BW], cc[:],
                                        ALU.mult)
                nc.vector.scalar_tensor_tensor(
                    cc[r0:r1], t1[r0:r1], 2.0, t2[r0:r1], ALU.mult, ALU.add)
                # h/2 = (sigmoid(2c) - .5) * so
                tct = work.tile([64, BW], rdt, tag=f"tct{c}")
                nc.scalar.activation(tct[:], cc[:], AF.Sigmoid, scale=2.0)
                nc.vector.scalar_tensor_tensor(
                    h_cur[r0:r1], tct[r0:r1], 0.5, sifog[r0:r1, 2 * BW:3 * BW],
                    ALU.subtract, ALU.mult)

            # --- emission: interleave phase-1 units with recurrence ---
            units = phase1_units()
            done_units = [False]

            def pump(n):
                if done_units[0]:
                    return
                for _ in range(n):
                    u = next(units, None)
                    if u is None:
                        done_units[0] = True
                        return
                    u()

            # stages 0..1 must exist before step 0; stage k before step 16k.
            # units per stage: 9 dma + 9 mm + 1 copy + 4 regroup = 23
            pump(2 * 23)
            for s in range(NSTEP):
                pump(3)
                for c in range(CH):
                    emit_step(s, c)

            pump(10**9)  # drain any leftover units (none expected)

            # --- FC + softmax (per chain) ---
            for c in range(CH):
                h_last = h_bufs[c][(NSTEP - 1) % 2]
                relu2 = relu2s[c]
                p1 = pg_pool.tile([16, BW], f32, tag=f"pg{c}")
                nc.tensor.matmul(p1[:], w1[:], h_last[:], start=True, stop=True)
                nc.scalar.activation(relu2[0:16, :], p1[:], AF.Relu)
                p2 = pg_pool.tile([BW, C], f32, tag=f"pg{c}")
                nc.tensor.matmul(p2[:], relu2[:], w2[:], start=True, stop=True)
                negmax = work.tile([BW, 1], f32, tag=f"negmax{c}")
                nc.vector.reduce_max(negmax[:], p2[:], mybir.AxisListType.X,
                                     negate=True)
                esum = work.tile([BW, 1], f32, tag=f"esum{c}")
                evals = work.tile([BW, C], f32, tag=f"evals{c}")
                nc.scalar.activation(evals[:], p2[:], AF.Exp, bias=negmax[:],
                                     accum_out=esum[:])
                rinv = work.tile([BW, 1], f32, tag=f"rinv{c}")
                nc.vector.reciprocal(rinv[:], esum[:])
                prob = work.tile([BW, C], f32, tag=f"prob{c}")
                nc.vector.tensor_scalar(prob[:], evals[:], rinv[:], None,
                                        ALU.mult)
                nc.sync.dma_start(out=out_d[c * BW:(c + 1) * BW, :],
                                  in_=prob[:])

    nc.compile()
    return nc


def _prep_inputs(inputs, x_dtype):
    x = inputs["x"]
    consts = build_host_constants(inputs, x_dtype, CFG["rec_dtype"])
    xdt = _np_dt(x_dtype)
    in_maps = []
    for g in range(NCORES):
        xc = x[g * BL:(g + 1) * BL]                      # [32, 200, 1086]
        xr = xc.reshape(BL, THI, TL, I).transpose(3, 1, 0, 2)  # [I,25,32,8]
        xf = np.ascontiguousarray(xr).reshape(I, NCOLS).astype(xdt)
        m = dict(x=xf, wproj=consts["W_proj"], w1=consts["W1e"],
                 w2=consts["W2"], sel=consts["SEL"])
        for t in TYPES:
            m[f"lhs_{t}"] = consts[f"lhsT_{t}"]
        in_maps.append(m)
    return in_maps


def kernel(**inputs):
    from concourse.bass_utils import run_bass_kernel_spmd

    x_dtype = CFG["x_dtype"]
    key = ("nc", x_dtype, CFG["nchains"], CFG["rec_dtype"])
    if key not in _BUILD_CACHE:
        _BUILD_CACHE[key] = build_bass(x_dtype, CFG["nchains"], CFG["rec_dtype"])
    nc = _BUILD_CACHE[key]
    in_maps = _prep_inputs(inputs, x_dtype)
    res = run_bass_kernel_spmd(nc, in_maps, list(range(NCORES)))
    out = np.concatenate([res.results[g]["out"] for g in range(NCORES)], axis=0)
    return out.astype(np.float32)
